# revision 1
# baseline (speedup 1.0000x reference)
"""Trainium2 Bass kernel for nn_Conv2d_Local (locally-connected conv, untied
weights).

Problem: x [B=128, 1, 560, 560]; weight [P*NF, 1, 28, 28] with P=39*39=1521
patch locations (stride 14, kernel 28), NF=64 filters; bias [P*NF, 1].
out[b, f*P+p] = sum_{kh,kw} x[b, i*14+kh, j*14+kw] * w[f*P+p, kh, kw] + bias.

Strategy: shard the 39 patch rows across 8 cores (5 rows each, row 39 padded).
Per patch p this is a GEMM patch[b, 784] @ w_p[784, 64]. The contraction is
chunked as 4 kh-groups of 7 rows x 2 kw-blocks of 14 cols (K=98 per chunk,
aligned to the stride so x chunks are shared between horizontally adjacent
patches). Adjacent patches' chunks that share the same x tile are paired into
one matmul of N=128 (two 64-wide weight halves -> two adjacent 64-col psum
slices), so each x tile is loaded stationary exactly once per patch row.

Host pre-permutes x and w into DMA-friendly layouts (pixel-major, batch
contiguous) so every DMA has large contiguous runs per partition; host also
adds the bias and reassembles the final output layout.
"""
import sys

if '/opt/trn_rl_repo' not in sys.path:
    sys.path.insert(0, '/opt/trn_rl_repo')

import numpy as np

B = 128
H = W = 560
KH = KW = 28
DH = DW = 14
NF = 64
OH = OW = 39
P = OH * OW
NCORES = 8
NROWS = 5          # patch rows per core (40 total, row 39 is padding)
NGROUPS = 12       # 7-row kh-groups per core: rows 2*ri .. 2*ri+3 per patch row
GROWS = 82         # global 7-row groups covering 574 (padded) x rows
XSLAB_BUFS = 5
WH_BUFS = 12    # weight pair tiles [98, 2, 1024], 10 live per row + prefetch
OROW_BUFS = 1
PSUM_BUFS = 8

_CACHE = {}


def build_program(repeats: int = 1, split_mm: bool = False, split_wdma: bool = False):
    import concourse.bacc as bacc
    import concourse.mybir as mybir
    from concourse.tile import TileContext

    f32 = mybir.dt.float32
    nc = bacc.Bacc("TRN2", target_bir_lowering=False, debug=False,
                   num_devices=NCORES)
    x_in = nc.dram_tensor("x", [NGROUPS, 98, 40, 128], f32, kind="ExternalInput")
    w_in = nc.dram_tensor("w", [NROWS, 5, 98, 4, 1024], f32, kind="ExternalInput")
    y_out = nc.dram_tensor("y", [NROWS, 128, OW * NF], f32, kind="ExternalOutput")

    with TileContext(nc) as tc:
        with tc.tile_pool(name="xslab", bufs=XSLAB_BUFS) as xpool, \
             tc.tile_pool(name="wh", bufs=WH_BUFS) as wpool, \
             tc.tile_pool(name="op", bufs=OROW_BUFS) as opool, \
             tc.tile_pool(name="ps", bufs=PSUM_BUFS, space="PSUM") as pspool:
            def block_slices(mrel, npat):
                """(weight col slice, psum col slice) for block mrel of a bank."""
                if mrel == 0:
                    return (0, 64), (0, 64)
                if mrel < npat:
                    return ((128 * mrel - 64, 128 * mrel + 64),
                            (64 * (mrel - 1), 64 * (mrel + 1)))
                return ((128 * npat - 64, 128 * npat),
                        (64 * (npat - 1), 64 * npat))

            for _rep in range(repeats):
                xslabs = {}

                def load_slab(gi):
                    # all input DMAs share the ACT HWDGE ring so emission
                    # order IS transfer order; slab loads are woven between
                    # weight DMAs at the points computed in the ring schedule
                    if gi not in xslabs:
                        t = xpool.tile([98, 40, 128], f32, tag="xslab",
                                       name=f"xs{gi}")
                        nc.scalar.dma_start(out=t, in_=x_in[gi])
                        xslabs[gi] = t
                    return xslabs[gi]

                # Each row runs 4 g-serial sub-passes (sub-pass g uses x slab
                # group 2ri+g only); weights stream as per-(bank, g-pair)
                # tiles so each row needs just 2 new slabs + 10 weight DMAs,
                # and compute can start as soon as slab0 + one weight tile
                # have landed.
                for ri in range(NROWS):
                    if ri == 0:
                        load_slab(0)
                    orow = opool.tile([128, OW * NF], f32, tag="orow",
                                      name=f"orow{ri}")
                    psrow = [pspool.tile([128, 512], f32, tag="ps",
                                         name=f"ps{ri}_{t5}")
                             for t5 in range(5)]
                    wtiles = {}
                    for g4 in range(4):
                        pair, gg = divmod(g4, 2)
                        slab = xslabs[2 * ri + g4]
                        for t5 in range(5):
                            p0 = 8 * t5
                            npat = 8 if t5 < 4 else 7
                            npc = npat * 128
                            if gg == 0:
                                wt = wpool.tile([98, 2, 1024], f32, tag="wh",
                                                name=f"wh{ri}_{t5}_{pair}")
                                if split_wdma:
                                    for q in range(2):
                                        nc.scalar.dma_start(
                                            out=wt[:, q, :npc],
                                            in_=w_in[ri, t5, :, 2 * pair + q, :npc])
                                else:
                                    nc.scalar.dma_start(
                                        out=wt[:, :, :npc],
                                        in_=w_in[ri, t5, :, 2 * pair: 2 * pair + 2, :npc])
                                wtiles[(t5, pair)] = wt
                                # woven slab loads (ring schedule):
                                if g4 == 0 and t5 == 2 and ri == 0:
                                    load_slab(1)
                                if g4 == 0 and t5 == 4:
                                    load_slab(2 * ri + 2)
                                if g4 == 2 and t5 == 0:
                                    load_slab(2 * ri + 3)
                            wt = wtiles[(t5, pair)]
                            for mrel in range(npat + 1):
                                m = p0 + mrel
                                wsl, osl = block_slices(mrel, npat)
                                start = (g4 == 0 and mrel == 0)
                                stop = (g4 == 3 and mrel == npat)
                                if split_mm and wsl[1] - wsl[0] == 128:
                                    nc.tensor.matmul(
                                        psrow[t5][:, osl[0]:osl[0] + 64],
                                        slab[:, m, :],
                                        wt[:, gg, wsl[0]:wsl[0] + 64],
                                        start=start, stop=False)
                                    nc.tensor.matmul(
                                        psrow[t5][:, osl[0] + 64:osl[1]],
                                        slab[:, m, :],
                                        wt[:, gg, wsl[0] + 64:wsl[1]],
                                        start=False, stop=stop)
                                else:
                                    nc.tensor.matmul(
                                        psrow[t5][:, osl[0]:osl[1]],
                                        slab[:, m, :],
                                        wt[:, gg, wsl[0]:wsl[1]],
                                        start=start, stop=stop)
                    for t5 in range(5):
                        npat = 8 if t5 < 4 else 7
                        nc.vector.tensor_copy(
                            out=orow[:, 512 * t5: 512 * t5 + npat * 64],
                            in_=psrow[t5][:, :npat * 64])
                    nc.gpsimd.dma_start(out=y_out[ri], in_=orow)
    nc.finalize()
    return nc


def _preprocess(x, weight):
    """Build per-core input maps from full x [B,1,560,560], weight [P*NF,1,28,28]."""
    x = np.ascontiguousarray(np.asarray(x, dtype=np.float32))
    weight = np.ascontiguousarray(np.asarray(weight, dtype=np.float32))

    # x -> pixel-major [574(pad), 560, 128], then 7-row slabs with partition
    # order (kh', kw'): [82, 98, 40, 128]
    xt = np.zeros((GROWS * 7, W, B), dtype=np.float32)
    xt[:H] = x[:, 0].transpose(1, 2, 0)
    x_dev = np.ascontiguousarray(
        xt.reshape(GROWS, 7, 40, 14, B).transpose(0, 1, 3, 2, 4)
    ).reshape(GROWS, 98, 40, 128)

    # weight rows are f*P + p; reshape kh=(g,kh'), kw=(delta,kw') and order as
    # [i, k=(kh',kw'), g, cols=(j, delta, f)]
    w6 = weight.reshape(NF, OH, OW, 4, 7, 2, 14)
    w_flat = np.ascontiguousarray(
        w6.transpose(1, 4, 6, 3, 2, 5, 0)  # [i, kh', kw', g, j, delta, f]
    ).reshape(OH, 98, 4, OW * 2 * NF)

    w_dev = np.zeros((NROWS * NCORES, 5, 98, 4, 1024), dtype=np.float32)
    for t5 in range(5):
        p0 = 8 * t5
        npat = 8 if t5 < 4 else 7
        w_dev[:OH, t5, :, :, :npat * 128] = \
            w_flat[:, :, :, 128 * p0: 128 * (p0 + npat)]

    in_maps = []
    for c in range(NCORES):
        in_maps.append({
            "x": np.ascontiguousarray(x_dev[10 * c: 10 * c + NGROUPS]),
            "w": np.ascontiguousarray(w_dev[NROWS * c: NROWS * (c + 1)]),
        })
    return in_maps


def _postprocess(results, bias):
    """results: list of per-core dicts with 'y' [NROWS, 128, OW*NF]."""
    y = np.stack([r["y"] for r in results])          # [8, 5, 128, 39*64]
    y = y.reshape(NCORES * NROWS, B, OW, NF)[:OH]    # [39, 128, 39, 64]
    out = np.ascontiguousarray(y.transpose(1, 3, 0, 2)).reshape(B, NF * P)
    out = out + np.asarray(bias, dtype=np.float32).reshape(1, NF * P)
    return out.reshape(B, NF * P, 1)


def kernel(x, weight, bias):
    from concourse.bass_utils import run_bass_kernel_spmd

    if "nc" not in _CACHE:
        _CACHE["nc"] = build_program()
    nc = _CACHE["nc"]
    in_maps = _preprocess(x, weight)
    res = run_bass_kernel_spmd(nc, in_maps, core_ids=list(range(NCORES)))
    return _postprocess(res.results, bias)



# revision 2
# speedup vs baseline: 2.6916x; 2.6916x over previous
"""Trainium2 Bass kernel for nn_Conv2d_Local (locally-connected conv, untied
weights).

Problem: x [B=128, 1, 560, 560]; weight [P*NF, 1, 28, 28] with P=39*39=1521
patch locations (stride 14, kernel 28), NF=64 filters; bias [P*NF, 1].
out[b, f*P+p] = sum_{kh,kw} x[b, i*14+kh, j*14+kw] * w[f*P+p, kh, kw] + bias.

Strategy: shard the 39 patch rows across 8 cores (5 rows each, row 39 padded).
Per patch p this is a GEMM patch[b, 784] @ w_p[784, 64]. The contraction is
chunked as 4 kh-groups of 7 rows x 2 kw-blocks of 14 cols (K=98 per chunk,
aligned to the stride so x chunks are shared between horizontally adjacent
patches). Adjacent patches' chunks that share the same x tile are paired into
one matmul of N=128 (two 64-wide weight halves -> two adjacent 64-col psum
slices), so each x tile is loaded stationary exactly once per patch row.

All device data is bf16 (inputs quantized host-side; psum accumulates fp32;
output stored bf16 and upcast on host), which quarters tensor-engine time and
halves HBM traffic vs fp32. Input DMAs ride both HWDGE rings: x slabs on the
SP ring (nc.sync), weights on the ACT ring (nc.scalar); output on SWDGE
(gpsimd). Host pre-permutes x and w into DMA-friendly layouts (pixel-major,
batch contiguous) so every DMA has large contiguous runs per partition; host
adds the bias and reassembles the final output layout in fp32.
"""
import sys

if '/opt/trn_rl_repo' not in sys.path:
    sys.path.insert(0, '/opt/trn_rl_repo')

import numpy as np

B = 128
H = W = 560
KH = KW = 28
DH = DW = 14
NF = 64
OH = OW = 39
P = OH * OW
NCORES = 8
NROWS = 5          # patch rows per core (40 total, row 39 is padding)
NGROUPS = 12       # 7-row kh-groups per core: rows 2*ri .. 2*ri+3 per patch row
GROWS = 82         # global 7-row groups covering 574 (padded) x rows
XSLAB_BUFS = 6
WH_BUFS = 20       # weight pair tiles [98, 2, 1024] bf16, 10 consumed per row
OROW_BUFS = 2
PSUM_BUFS = 8

_CACHE = {}


def build_program(repeats: int = 1):
    import concourse.bacc as bacc
    import concourse.mybir as mybir
    from concourse.tile import TileContext

    f32 = mybir.dt.float32
    bf16 = mybir.dt.bfloat16
    nc = bacc.Bacc("TRN2", target_bir_lowering=False, debug=False,
                   num_devices=NCORES)
    x_in = nc.dram_tensor("x", [NGROUPS, 98, 40, 128], bf16, kind="ExternalInput")
    w_in = nc.dram_tensor("w", [NROWS, 5, 98, 4, 1024], bf16, kind="ExternalInput")
    y_out = nc.dram_tensor("y", [NROWS, 128, OW * NF], bf16, kind="ExternalOutput")

    with TileContext(nc) as tc:
        with tc.tile_pool(name="xslab", bufs=XSLAB_BUFS) as xpool, \
             tc.tile_pool(name="wh", bufs=WH_BUFS) as wpool, \
             tc.tile_pool(name="op", bufs=OROW_BUFS) as opool, \
             tc.tile_pool(name="ps", bufs=PSUM_BUFS, space="PSUM") as pspool:
            def block_slices(mrel, npat):
                """(weight col slice, psum col slice) for block mrel of a bank."""
                if mrel == 0:
                    return (0, 64), (0, 64)
                if mrel < npat:
                    return ((128 * mrel - 64, 128 * mrel + 64),
                            (64 * (mrel - 1), 64 * (mrel + 1)))
                return ((128 * npat - 64, 128 * npat),
                        (64 * (npat - 1), 64 * npat))

            for _rep in range(repeats):
                xslabs = {}

                def load_slab(gi):
                    # x slabs ride the SP HWDGE ring (nc.sync), independent of
                    # the weight stream on the ACT ring; emit in consumption
                    # order so the per-ring FIFO matches demand
                    if gi not in xslabs:
                        t = xpool.tile([98, 40, 128], bf16, tag="xslab",
                                       name=f"xs{gi}")
                        nc.sync.dma_start(out=t, in_=x_in[gi])
                        xslabs[gi] = t
                    return xslabs[gi]

                for ri in range(NROWS):
                    # current row's slabs plus lookahead into the next row
                    for g in range(2 * ri, min(2 * ri + 6, 2 * NROWS + 2)):
                        load_slab(g)
                    orow = opool.tile([128, OW * NF], bf16, tag="orow",
                                      name=f"orow{ri}")
                    psrow = [pspool.tile([128, 512], f32, tag="ps",
                                         name=f"ps{ri}_{t5}")
                             for t5 in range(5)]
                    wtiles = {}
                    for g4 in range(4):
                        pair, gg = divmod(g4, 2)
                        slab = xslabs[2 * ri + g4]
                        for t5 in range(5):
                            p0 = 8 * t5
                            npat = 8 if t5 < 4 else 7
                            npc = npat * 128
                            if gg == 0:
                                wt = wpool.tile([98, 2, 1024], bf16, tag="wh",
                                                name=f"wh{ri}_{t5}_{pair}")
                                nc.scalar.dma_start(
                                    out=wt[:, :, :npc],
                                    in_=w_in[ri, t5, :, 2 * pair: 2 * pair + 2, :npc])
                                wtiles[(t5, pair)] = wt
                            wt = wtiles[(t5, pair)]
                            for mrel in range(npat + 1):
                                m = p0 + mrel
                                wsl, osl = block_slices(mrel, npat)
                                start = (g4 == 0 and mrel == 0)
                                stop = (g4 == 3 and mrel == npat)
                                nc.tensor.matmul(
                                    psrow[t5][:, osl[0]:osl[1]],
                                    slab[:, m, :],
                                    wt[:, gg, wsl[0]:wsl[1]],
                                    start=start, stop=stop)
                    for t5 in range(5):
                        npat = 8 if t5 < 4 else 7
                        nc.vector.tensor_copy(
                            out=orow[:, 512 * t5: 512 * t5 + npat * 64],
                            in_=psrow[t5][:, :npat * 64])
                    nc.gpsimd.dma_start(out=y_out[ri], in_=orow)
    nc.finalize()
    return nc


def _preprocess(x, weight):
    """Build per-core bf16 input maps from full x [B,1,560,560],
    weight [P*NF,1,28,28]."""
    import ml_dtypes
    bf16 = ml_dtypes.bfloat16

    x = np.asarray(x, dtype=np.float32).astype(bf16)
    weight = np.asarray(weight, dtype=np.float32).astype(bf16)

    # x -> pixel-major [574(pad), 560, 128], then 7-row slabs with partition
    # order (kh', kw'): [82, 98, 40, 128]
    xt = np.zeros((GROWS * 7, W, B), dtype=bf16)
    xt[:H] = x[:, 0].transpose(1, 2, 0)
    x_dev = np.ascontiguousarray(
        xt.reshape(GROWS, 7, 40, 14, B).transpose(0, 1, 3, 2, 4)
    ).reshape(GROWS, 98, 40, 128)

    # weight rows are f*P + p; reshape kh=(g,kh'), kw=(delta,kw') and order as
    # [i, k=(kh',kw'), g, cols=(j, delta, f)]
    w6 = weight.reshape(NF, OH, OW, 4, 7, 2, 14)
    w_flat = np.ascontiguousarray(
        w6.transpose(1, 4, 6, 3, 2, 5, 0)  # [i, kh', kw', g, j, delta, f]
    ).reshape(OH, 98, 4, OW * 2 * NF)

    w_dev = np.zeros((NROWS * NCORES, 5, 98, 4, 1024), dtype=bf16)
    for t5 in range(5):
        p0 = 8 * t5
        npat = 8 if t5 < 4 else 7
        w_dev[:OH, t5, :, :, :npat * 128] = \
            w_flat[:, :, :, 128 * p0: 128 * (p0 + npat)]

    in_maps = []
    for c in range(NCORES):
        in_maps.append({
            "x": np.ascontiguousarray(x_dev[10 * c: 10 * c + NGROUPS]),
            "w": np.ascontiguousarray(w_dev[NROWS * c: NROWS * (c + 1)]),
        })
    return in_maps


def _postprocess(results, bias):
    """results: list of per-core dicts with 'y' [NROWS, 128, OW*NF] bf16."""
    y = np.stack([np.asarray(r["y"], dtype=np.float32) for r in results])
    y = y.reshape(NCORES * NROWS, B, OW, NF)[:OH]    # [39, 128, 39, 64]
    out = np.ascontiguousarray(y.transpose(1, 3, 0, 2)).reshape(B, NF * P)
    out = out + np.asarray(bias, dtype=np.float32).reshape(1, NF * P)
    return out.reshape(B, NF * P, 1)


def kernel(x, weight, bias):
    from concourse.bass_utils import run_bass_kernel_spmd

    if "nc" not in _CACHE:
        _CACHE["nc"] = build_program()
    nc = _CACHE["nc"]
    in_maps = _preprocess(x, weight)
    res = run_bass_kernel_spmd(nc, in_maps, core_ids=list(range(NCORES)))
    return _postprocess(res.results, bias)
